# revision 16
# baseline (speedup 1.0000x reference)
"""Trainium2 Bass kernel for the AttentionBlock problem.

Math (per batch b):
    qp = l2norm(q @ W.T + bias); kp = l2norm(k @ W.T + bias)
    attn = softmax(scale_att * qp @ kp.T - 1000 * mask)
    out  = att_wt * (attn @ v) + org_wt * idt
Returns (out, attn) like the reference.

Sharding: pure data-parallel — batch b -> NeuronCore b (B == 8 == n_cores).
No collectives. Each core runs the identical program on its own batch.
"""

import numpy as np

import concourse.bass as bass
import concourse.bacc as bacc
import concourse.mybir as mybir
from concourse import masks
from concourse.tile import TileContext
from concourse.bass_utils import run_bass_kernel_spmd

F32 = mybir.dt.float32
BF16 = mybir.dt.bfloat16
FP8 = mybir.dt.float8e4  # e4m3

AF = mybir.ActivationFunctionType
ALU = mybir.AluOpType

P = 128
NBLK = 512  # moving-free-dim block (one PSUM bank of f32)

# full-size problem constants (hardcoded per harness rules)
FULL_NQ, FULL_NS, FULL_C, FULL_D = 4096, 4096, 2048, 512
N_CORES = 8

# fp8 DoubleRow perf mode for the AV matmul (2 ns-tiles per instruction).
# Off by default: not yet validated on hardware (CoreSim-validated only).
USE_DOUBLE_ROW = False


def build_nc(NQ=FULL_NQ, NS=FULL_NS, C=FULL_C, D=FULL_D):
    CT = C // P        # contraction tiles (and d-tiles of the projection)
    NTQ = NQ // P      # query tiles
    NTS = NS // P      # support tiles
    NSB = NS // NBLK   # support blocks of 512
    NQB = NQ // NBLK   # query blocks of 512 (for q projection)
    assert D <= NBLK

    nc = bacc.Bacc()
    k_p = nc.declare_dram_parameter("k", [NS, C], F32, isOutput=False)
    v_p = nc.declare_dram_parameter("v", [NS, D], F32, isOutput=False)
    q_p = nc.declare_dram_parameter("q", [NQ, C], F32, isOutput=False)
    idt_p = nc.declare_dram_parameter("idt", [NQ, D], F32, isOutput=False)
    m_p = nc.declare_dram_parameter("mask", [1, NS], F32, isOutput=False)
    W_p = nc.declare_dram_parameter("W", [C, C], F32, isOutput=False)
    b_p = nc.declare_dram_parameter("b", [1, C], F32, isOutput=False)
    sa_p = nc.declare_dram_parameter("sa", [1, 1], F32, isOutput=False)
    awt_p = nc.declare_dram_parameter("awt", [1, 1], F32, isOutput=False)
    owt_p = nc.declare_dram_parameter("owt", [1, 1], F32, isOutput=False)
    out_p = nc.declare_dram_parameter("out", [NQ, D], F32, isOutput=True)
    attn_p = nc.declare_dram_parameter("attn", [NQ, NS], F32, isOutput=True)

    with TileContext(nc) as tc:
        with (
            tc.tile_pool(name="const", bufs=1) as constp,
            tc.tile_pool(name="dram", bufs=1, space="DRAM") as dramp,
        ):
            # ---------------- constants ----------------
            ident = constp.tile([P, P], BF16)
            masks.make_identity(nc, ident[:])
            ones_r = constp.tile([1, P], BF16)   # K=1 row of ones (broadcast)
            nc.gpsimd.memset(ones_r[:], 1.0)
            ones_c = constp.tile([P, 1], BF16)   # K=P ones column (partition sum)
            nc.gpsimd.memset(ones_c[:], 1.0)
            ones_rf = constp.tile([1, P], F32)
            nc.gpsimd.memset(ones_rf[:], 1.0)

            b_sb = constp.tile([P, CT], F32)
            nc.sync.dma_start(
                out=b_sb[:], in_=b_p[:, :].rearrange("a (t p) -> (a p) t", p=P)
            )

            sa_sb = constp.tile([1, 1], F32)
            nc.sync.dma_start(out=sa_sb[:], in_=sa_p[:, :])
            awt_sb = constp.tile([1, 1], F32)
            nc.sync.dma_start(out=awt_sb[:], in_=awt_p[:, :])
            owt_sb = constp.tile([1, 1], F32)
            nc.sync.dma_start(out=owt_sb[:], in_=owt_p[:, :])

            # broadcast scalars to [P, 1] via K=1 outer product
            sa_b = constp.tile([P, 1], F32)
            awt_b = constp.tile([P, 1], F32)
            owt_b = constp.tile([P, 1], F32)
            with tc.tile_pool(name="scps", bufs=1, space="PSUM") as scpsp:
                for src, dst in ((sa_sb, sa_b), (awt_sb, awt_b), (owt_sb, owt_b)):
                    ps = scpsp.tile([P, 1], F32, tag="sc")
                    nc.tensor.matmul(ps[:], ones_rf[:], src[:], start=True, stop=True)
                    nc.scalar.copy(dst[:], ps[:])

            # mask row folded into the logits matmul as a K=1 rank-1 term:
            # ek[s] = mask[s] * (-1000 / scale_att); exp later rescales by sa.
            rsa = constp.tile([1, 1], F32)
            nc.vector.reciprocal(rsa[:], sa_sb[:])
            c0 = constp.tile([1, 1], F32)
            nc.scalar.mul(c0[:], rsa[:], -1000.0)
            ek = constp.tile([1, NS], BF16)
            maskb = constp.tile([P, NS], BF16)
            with (
                tc.tile_pool(name="mload", bufs=1) as mloadp,
                tc.tile_pool(name="mps", bufs=2, space="PSUM") as mpsp,
            ):
                m_sb = mloadp.tile([1, NS], F32)
                nc.sync.dma_start(out=m_sb[:], in_=m_p[:, :])
                nc.scalar.mul(ek[:], m_sb[:], c0[:, :])
                for blk in range(NSB):
                    mp = mpsp.tile([P, NBLK], F32, tag="mp")
                    nc.tensor.matmul(
                        mp[:], ones_r[:], ek[:, blk * NBLK:(blk + 1) * NBLK],
                        start=True, stop=True,
                    )
                    nc.scalar.copy(maskb[:, blk * NBLK:(blk + 1) * NBLK], mp[:])

            # scratch for the normalized, transposed projections (bf16)
            kpT_scratch = dramp.tile([CT, P, NS], BF16)
            qpT_scratch = dramp.tile([NTQ, P, CT, P], BF16)

            # ---------------- phase 0: W -> WT (bf16, resident) ----------------
            with tc.tile_pool(name="wt", bufs=1) as wtp:
                wt_sb = wtp.tile([P, CT, C], BF16)  # [c%P, c//P, d]
                with (
                    tc.tile_pool(name="wload", bufs=2) as wloadp,
                    tc.tile_pool(name="wpsum", bufs=4, space="PSUM") as wpsump,
                ):
                    for dt in range(CT):
                        wrow = wloadp.tile([P, C], F32, tag="wrow")
                        nc.sync.dma_start(
                            out=wrow[:], in_=W_p[dt * P:(dt + 1) * P, :]
                        )
                        wrow_b = wloadp.tile([P, C], BF16, tag="wrowb")
                        nc.scalar.copy(wrow_b[:], wrow[:])
                        for ct in range(CT):
                            tp = wpsump.tile([P, P], BF16, tag="wtp")
                            nc.tensor.transpose(
                                tp[:], wrow_b[:, ct * P:(ct + 1) * P], ident[:]
                            )
                            nc.vector.tensor_copy(
                                wt_sb[:, ct, dt * P:(dt + 1) * P], tp[:]
                            )

                # ---------------- phases 1+2: project q, then k ----------------
                with (
                    tc.tile_pool(name="xload", bufs=2) as xloadp,
                    tc.tile_pool(name="xt", bufs=2) as xtp,
                    tc.tile_pool(name="pre", bufs=2) as prep,
                    tc.tile_pool(name="sq", bufs=4) as sqp,
                    tc.tile_pool(name="pout", bufs=2) as poutp,
                    tc.tile_pool(name="nrm", bufs=2) as nrmp,
                    tc.tile_pool(name="ppsum", bufs=4, space="PSUM") as ppsump,
                    tc.tile_pool(name="tpsum", bufs=2, space="PSUM") as tpsump,
                    tc.tile_pool(name="spsum", bufs=1, space="PSUM") as spsump,
                ):
                    def project(x_p, nblocks, is_q):
                        for blk in range(nblocks):
                            # load 4x128 rows, cast bf16, transpose to [c, n]
                            xt = xtp.tile([P, CT, NBLK], BF16, tag="xt")
                            for s in range(4):
                                r0 = blk * NBLK + s * P
                                xrow = xloadp.tile([P, C], F32, tag="xrow")
                                nc.sync.dma_start(
                                    out=xrow[:], in_=x_p[r0:r0 + P, :]
                                )
                                xrow_b = xloadp.tile([P, C], BF16, tag="xrowb")
                                nc.scalar.copy(xrow_b[:], xrow[:])
                                for ct in range(CT):
                                    tp = tpsump.tile([P, P], BF16, tag="tp")
                                    nc.tensor.transpose(
                                        tp[:],
                                        xrow_b[:, ct * P:(ct + 1) * P],
                                        ident[:],
                                    )
                                    nc.vector.tensor_copy(
                                        xt[:, ct, s * P:(s + 1) * P], tp[:]
                                    )
                            # project: pre[d, n] = W.T-contract + bias; also
                            # squares + partition-sum (interleaved per dt)
                            pre = prep.tile([P, CT, NBLK], BF16, tag="pre")
                            ssps = spsump.tile([1, NBLK], F32, tag="ss")
                            for dt in range(CT):
                                ps = ppsump.tile([P, NBLK], F32, tag="pp")
                                for ct in range(CT):
                                    nc.tensor.matmul(
                                        ps[:],
                                        wt_sb[:, ct, dt * P:(dt + 1) * P],
                                        xt[:, ct, :],
                                        start=(ct == 0),
                                        stop=(ct == CT - 1),
                                    )
                                bias = b_sb[:, dt:dt + 1]
                                nc.scalar.activation(
                                    pre[:, dt, :], ps[:], AF.Identity, bias=bias
                                )
                                sqt = sqp.tile([P, NBLK], BF16, tag="sq")
                                nc.scalar.activation(
                                    sqt[:], ps[:], AF.Square, bias=bias
                                )
                                nc.tensor.matmul(
                                    ssps[:],
                                    ones_c[:],
                                    sqt[:],
                                    start=(dt == 0),
                                    stop=(dt == CT - 1),
                                )
                            nrm = nrmp.tile([1, NBLK], F32, tag="nrm")
                            nc.scalar.sqrt(nrm[:], ssps[:])
                            rn = nrmp.tile([1, NBLK], F32, tag="rn")
                            nc.vector.reciprocal(rn[:], nrm[:])
                            rnb = nrmp.tile([1, NBLK], BF16, tag="rnb")
                            nc.scalar.copy(rnb[:], rn[:])
                            rbps = spsump.tile([P, NBLK], F32, tag="rb")
                            nc.tensor.matmul(
                                rbps[:], ones_r[:], rnb[:], start=True, stop=True
                            )
                            # normalize and write to scratch
                            for dt in range(CT):
                                o = poutp.tile([P, NBLK], BF16, tag="po")
                                nc.vector.tensor_mul(o[:], pre[:, dt, :], rbps[:])
                                if is_q:
                                    for s in range(4):
                                        t = blk * 4 + s
                                        nc.sync.dma_start(
                                            out=qpT_scratch[t, :, dt, :],
                                            in_=o[:, s * P:(s + 1) * P],
                                        )
                                else:
                                    nc.sync.dma_start(
                                        out=kpT_scratch[
                                            dt, :, blk * NBLK:(blk + 1) * NBLK
                                        ],
                                        in_=o[:],
                                    )

                    project(q_p, NQB, True)
                    project(k_p, NSB, False)

            # ---------------- phase 3: attention ----------------
            with (
                tc.tile_pool(name="res", bufs=1) as resp,
                tc.tile_pool(name="qld", bufs=2) as qldp,
                tc.tile_pool(name="exp", bufs=1) as expp,
                tc.tile_pool(name="expt", bufs=1) as exptp,
                tc.tile_pool(name="ach", bufs=2) as achp,
                tc.tile_pool(name="lgm", bufs=2) as lgmp,
                tc.tile_pool(name="io3", bufs=2) as io3p,
                tc.tile_pool(name="rs", bufs=2) as rsp,
                tc.tile_pool(name="lps", bufs=4, space="PSUM") as lpsp,
                tc.tile_pool(name="tps", bufs=2, space="PSUM") as tpsp,
                tc.tile_pool(name="aps", bufs=2, space="PSUM") as apsp,
            ):
                kpt = resp.tile([P, CT, NS], BF16)
                for dt in range(CT):
                    nc.sync.dma_start(out=kpt[:, dt, :], in_=kpT_scratch[dt, :, :])
                v_sb = resp.tile([P, NTS, D], FP8)
                with tc.tile_pool(name="vload", bufs=2) as vloadp:
                    for i in range(NTS):
                        vr = vloadp.tile([P, D], F32, tag="vr")
                        nc.sync.dma_start(out=vr[:], in_=v_p[i * P:(i + 1) * P, :])
                        nc.scalar.copy(v_sb[:, i, :], vr[:])

                for t in range(NTQ):
                    qpt = qldp.tile([P, CT, P], BF16, tag="qpt")
                    nc.sync.dma_start(out=qpt[:], in_=qpT_scratch[t, :, :, :])
                    exp_row = expp.tile([P, NS], BF16, tag="exp")
                    rsum8 = rsp.tile([P, NSB], F32, tag="rs8")
                    for blk in range(NSB):
                        c0b = blk * NBLK
                        ps = lpsp.tile([P, NBLK], F32, tag="lg")
                        for dt in range(CT):
                            nc.tensor.matmul(
                                ps[:],
                                qpt[:, dt, :],
                                kpt[:, dt, c0b:c0b + NBLK],
                                start=(dt == 0),
                                stop=(dt == CT - 1),
                            )
                        # additive validity mask (pre-scaled by 1/sa)
                        lgm = lgmp.tile([P, NBLK], F32, tag="lgm")
                        nc.vector.tensor_add(
                            lgm[:], ps[:], maskb[:, c0b:c0b + NBLK]
                        )
                        nc.scalar.activation(
                            exp_row[:, c0b:c0b + NBLK], lgm[:], AF.Exp,
                            scale=sa_b[:, :], accum_out=rsum8[:, blk:blk + 1],
                        )
                    rsum = rsp.tile([P, 1], F32, tag="rst")
                    nc.vector.reduce_sum(
                        rsum[:], rsum8[:], axis=mybir.AxisListType.X
                    )
                    rrec = rsp.tile([P, 1], F32, tag="rrec")
                    nc.vector.reciprocal(rrec[:], rsum[:])
                    # attn output = exp * (1/rowsum)
                    for blk in range(NSB):
                        c0b = blk * NBLK
                        ach = achp.tile([P, NBLK], F32, tag="ach")
                        nc.scalar.mul(
                            ach[:], exp_row[:, c0b:c0b + NBLK], rrec[:, :]
                        )
                        nc.sync.dma_start(
                            out=attn_p[t * P:(t + 1) * P, c0b:c0b + NBLK],
                            in_=ach[:],
                        )
                    # transpose exp -> [s, nq] (fp8) for the AV matmul
                    et = exptp.tile([P, NTS, P], FP8, tag="et")
                    for i in range(NTS):
                        tp = tpsp.tile([P, P], BF16, tag="tp3")
                        nc.tensor.transpose(
                            tp[:], exp_row[:, i * P:(i + 1) * P], ident[:]
                        )
                        nc.vector.tensor_copy(et[:, i, :], tp[:])
                    # AV (fp8; DoubleRow packs two ns-tiles per matmul)
                    av = apsp.tile([P, D], F32, tag="av")
                    if USE_DOUBLE_ROW:
                        for i in range(NTS // 2):
                            nc.tensor.matmul(
                                av[:],
                                et[:, 2 * i:2 * i + 2, :],
                                v_sb[:, 2 * i:2 * i + 2, :],
                                start=(i == 0), stop=(i == NTS // 2 - 1),
                                perf_mode=mybir.MatmulPerfMode.DoubleRow,
                            )
                    else:
                        for i in range(NTS):
                            nc.tensor.matmul(
                                av[:], et[:, i, :], v_sb[:, i, :],
                                start=(i == 0), stop=(i == NTS - 1),
                            )
                    # out = att_wt * av / rowsum + org_wt * idt
                    idt_sb = io3p.tile([P, D], F32, tag="idt")
                    nc.sync.dma_start(
                        out=idt_sb[:], in_=idt_p[t * P:(t + 1) * P, :]
                    )
                    comb = rsp.tile([P, 1], F32, tag="comb")
                    nc.vector.tensor_mul(comb[:], rrec[:], awt_b[:])
                    idt_s = io3p.tile([P, D], F32, tag="idts")
                    nc.vector.tensor_scalar_mul(idt_s[:], idt_sb[:], owt_b[:, :])
                    out_sb = io3p.tile([P, D], F32, tag="outt")
                    nc.vector.scalar_tensor_tensor(
                        out_sb[:], av[:], comb[:, :], idt_s[:],
                        op0=ALU.mult, op1=ALU.add,
                    )
                    nc.sync.dma_start(
                        out=out_p[t * P:(t + 1) * P, :], in_=out_sb[:]
                    )
    nc.compile()
    return nc


_CACHE = {}
LAST_RESULT = None
TRACE = False


def run_shards(nc, shard_maps, trace=False):
    """Run the prebuilt graph on len(shard_maps) cores; returns BassKernelResults."""
    return run_bass_kernel_spmd(
        nc, shard_maps, core_ids=list(range(len(shard_maps))), trace=trace
    )


def kernel(k, v, q, idt, s_valid_mask, W, b, scale_att, att_wt_w, org_wt_w):
    global LAST_RESULT
    k = np.asarray(k, np.float32)
    v = np.asarray(v, np.float32)
    q = np.asarray(q, np.float32)
    idt = np.asarray(idt, np.float32)
    s_valid_mask = np.asarray(s_valid_mask, np.float32)
    W = np.ascontiguousarray(np.asarray(W, np.float32))
    b = np.asarray(b, np.float32).reshape(1, -1)
    sa = np.asarray(scale_att, np.float32).reshape(1, 1)
    awt = np.asarray(att_wt_w, np.float32).reshape(1, 1)
    owt = np.asarray(org_wt_w, np.float32).reshape(1, 1)

    B = k.shape[0]
    assert B == N_CORES, f"expected B == {N_CORES}, got {B}"

    if "nc" not in _CACHE:
        _CACHE["nc"] = build_nc()
    nc = _CACHE["nc"]

    in_maps = []
    for bi in range(B):
        in_maps.append({
            "k": np.ascontiguousarray(k[bi]),
            "v": np.ascontiguousarray(v[bi]),
            "q": np.ascontiguousarray(q[bi]),
            "idt": np.ascontiguousarray(idt[bi]),
            "mask": np.ascontiguousarray(s_valid_mask[bi].reshape(1, -1)),
            "W": W,
            "b": b,
            "sa": sa,
            "awt": awt,
            "owt": owt,
        })
    res = run_shards(nc, in_maps, trace=TRACE)
    LAST_RESULT = res
    out = np.stack([res.results[i]["out"] for i in range(B)])
    attn = np.stack([res.results[i]["attn"] for i in range(B)])
    return out, attn
